# revision 1
# baseline (speedup 1.0000x reference)
"""Trainium2 Bass kernel for nn_ExpansionContrastModule.

Sharding: 8 cores = 4 batches x 2 H-halves (80 rows each). Bottom halves are
row-flipped on the host (conv weights H-flipped to match) so that image-pad
rows always sit at shard-top; the dwconv-product-sort stage is flip-invariant
because it only depends on the +/-v_m offset pairs and the sort is
permutation-invariant over directions.

Within a core the 80 owned rows split into two 40-row sub-halves A/B mapped to
SBUF partitions 0:64 / 64:128 (64 channels each), computed in lockstep:
conv matmuls use block-diagonal duplicated weights (K=128, M=128) and all
vector ops run fully packed [128, N].

Restructured dwconv-contrast: with o1_m(p) = x(p) - x(p + v_m),
  o_m(p) = o1_m(p) * (x(p) - x(p - v_m)) = -o1_m(p) * o1_m(p - v_m),
so only 4 difference maps per branch are needed; the negation is absorbed by
using adjusted scales s'[g, j] = -scales3[g, 3-j] on the sorted products
t_m = o1_m(p) * o1_m(p - v_m).

Geometry (per half, local coords): shard = 100 rows (10 pad/halo + 80 + 10
halo). Owned rows = shard 10..89; A owns 10..49, B owns 50..89.
x0 (in_conv out): 60 rows per half, row r <-> shard r (A) / 40+r (B).
x_k (branch conv out): rows_x = 40+2d rows, row i <-> shard 10-d+i (A),
50-d+i (B); width Wx = 160+4d with real cols at [2d, 2d+160).
Shard rows 0..9 are always image-pad (flip trick) -> x0 rows 0..9 and
x_k rows 0..d-1 of the A half are memset to zero.
"""

import os

os.environ.setdefault("MYCRO_LOCAL_CACHE", "1")

import numpy as np
import ml_dtypes

import concourse.bass as bass
import concourse.bacc as bacc
import concourse.mybir as mybir
from concourse.tile import TileContext
from concourse import bass_utils

W = 160
SH = 100          # shard rows
HALO = 10
OWNH = 40         # owned rows per half
C = 256
CH = 64           # trunk channels
RB = 10           # post-stage block rows
RF = 2            # final-stage block rows
KS = [1, 3, 5, 7]         # branch conv kernel sizes
DIL = [1, 3, 5, 7]        # branch dwconv dilations (= shift)
TAP_OFF = [0, 1, 10, 35]  # cumulative tap offsets into wtap
NTAP = 84

F32 = mybir.dt.float32
F32R = mybir.dt.float32r
BF16 = mybir.dt.bfloat16
ALU = mybir.AluOpType
ACTF = mybir.ActivationFunctionType


def build_nc(dbg=False):
    nc = bacc.Bacc("TRN2", target_bir_lowering=False, debug=False,
                   enable_asserts=False, num_devices=8)

    din = {}
    def dram(name, shape, dt, kind="ExternalInput"):
        t = nc.dram_tensor(name, list(shape), dt, kind=kind)
        din[name] = t
        return t.ap()

    cen = [dram(f"cen{c}", (128, SH * W), F32) for c in range(2)]
    cenb = [dram(f"cenb{c}", (128, SH * W), BF16) for c in range(2)]
    win = dram("win", (128, 128), BF16)
    wtap = dram("wtap", (128, NTAP * 128), BF16)
    wbc = dram("wbc", (128, 128), BF16)
    wfc = dram("wfc", (128, 2), BF16)
    ones1 = dram("ones1", (1, 128), BF16)
    bin_ = dram("bin", (128, 1), F32)
    cb = dram("cb", (128, 4), F32)
    sadj = dram("sadj", (128, 16), F32)
    bnsc = dram("bnsc", (128, 1), F32)
    bnbi = dram("bnbi", (128, 1), F32)
    fcb = dram("fcb", (1, 1), F32)
    outs = [dram(f"out{c}", (128, OWNH * 2 * W), F32, kind="ExternalOutput")
            for c in range(2)]
    if dbg:
        dbg_x0 = dram("dbg_x0", (128, 60 * (W + 6)), BF16,
                      kind="ExternalOutput")
        dbg_x = [dram(f"dbg_b{k}", (128, (OWNH + 2 * d) * (W + 4 * d)), BF16,
                      kind="ExternalOutput") for k, d in enumerate(DIL)]
        dbg_v = [dram(f"dbg_{nm}", (128, OWNH * W), BF16,
                      kind="ExternalOutput") for nm in ("vmax", "vsum")]

    with TileContext(nc) as tc:
        with tc.tile_pool(name="cpool", bufs=1) as cp, \
             tc.tile_pool(name="inpool", bufs=2) as ip, \
             tc.tile_pool(name="x0pool", bufs=1) as x0p, \
             tc.tile_pool(name="xpool", bufs=2) as xp, \
             tc.tile_pool(name="o1pool", bufs=2) as o1p, \
             tc.tile_pool(name="tpool", bufs=1) as tp, \
             tc.tile_pool(name="ypool", bufs=2) as yp, \
             tc.tile_pool(name="vpool", bufs=1) as vp, \
             tc.tile_pool(name="fpool", bufs=2) as fp, \
             tc.tile_pool(name="pspool", bufs=1, space="PSUM") as pp:

            # ---- constants to SBUF
            win_s = cp.tile_from(win, name="win_s")
            wtap_s = cp.tile_from(wtap, name="wtap_s")
            wbc_s = cp.tile_from(wbc, name="wbc_s")
            wfc_s = cp.tile_from(wfc, name="wfc_s")
            ones_s = cp.tile_from(ones1, name="ones_s")
            bin_s = cp.tile_from(bin_, name="bin_s")
            cb_s = cp.tile_from(cb, name="cb_s")
            sadj_s = cp.tile_from(sadj, name="sadj_s")
            bnsc_s = cp.tile_from(bnsc, name="bnsc_s")
            bnbi_s = cp.tile_from(bnbi, name="bnbi_s")
            fcb_s = cp.tile_from(fcb, name="fcb_s")

            # ---- Phase A: in_conv -> x0 [128, 60*166] bf16
            WP0 = W + 6           # x0 width, real cols at [3, 163)
            X0R = 60
            x0 = x0p.tile([128, X0R * WP0], BF16, name="x0")
            nc.gpsimd.memset(x0[:, :], 0.0)
            RT = 3                # rows per in_conv tile
            for t0 in range(0, X0R, RT):
                n = RT * W
                ps = pp.tile([128, n], F32, tag="cvps", bufs=2, name="ps_in")
                ctiles = []
                for h, base in ((0, 0), (1, 40)):
                    for c in range(2):
                        ct = ip.tile([128, n], BF16, tag=f"cen{h}{c}",
                                     name=f"ct{h}{c}")
                        nc.gpsimd.dma_start(
                            out=ct[:, :],
                            in_=cenb[c][:, (base + t0) * W:(base + t0) * W + n])
                        ctiles.append((h, c, ct))
                for h, c, ct in ctiles:
                    nc.tensor.matmul(
                        ps[h * 64:h * 64 + 64, :],
                        lhsT=win_s[:, c * 64:c * 64 + 64],
                        rhs=ct[:, :],
                        start=(c == 0), stop=(c == 1))
                # copy + bias -> x0 (strided rows), cast bf16
                nc.scalar.activation(
                    x0[:, :].rearrange("p (r w) -> p r w", w=WP0)
                    [:, t0:t0 + RT, 3:3 + W],
                    ps[:, :].rearrange("p (r w) -> p r w", w=W),
                    ACTF.Identity, bias=bin_s[:, 0:1])
            # zero image-pad rows of A half (shard rows 0..9)
            nc.gpsimd.memset(x0[0:64, 0:HALO * WP0], 0.0)
            if dbg:
                nc.sync.dma_start(out=dbg_x0[:, :], in_=x0[:, :])

            # ---- vmax / vsum accumulators [128, 40*160] bf16
            vmax = vp.tile([128, OWNH * W], BF16, name="vmax")
            vsum = vp.tile([128, OWNH * W], BF16, name="vsum")

            x0v = x0[:, :].rearrange("p (r w) -> p r w", w=WP0)

            for k in range(4):
                d = DIL[k]
                ksz = KS[k]
                pad = ksz // 2
                rows_x = OWNH + 2 * d
                Wx = W + 4 * d
                xk = xp.tile([128, rows_x * Wx], BF16, tag="x", name=f"x{k}")
                xv = xk[:, :].rearrange("p (r w) -> p r w", w=Wx)
                # zero col pads: strided [rows, 4d] at col 2d+W covers right
                # pad of each row + left pad of next; plus row0 left pad.
                nc.gpsimd.memset(xk[:, 0:2 * d], 0.0)
                nc.gpsimd.memset(
                    xk[:, 2 * d + W:2 * d + W + (rows_x - 1) * Wx]
                    .rearrange("p (r w) -> p r w", w=Wx)[:, :, 0:4 * d], 0.0)
                nc.gpsimd.memset(
                    xk[:, (rows_x - 1) * Wx + 2 * d + W:rows_x * Wx], 0.0)

                # branch conv: psum tiles of RT rows
                for rt in range(0, rows_x, RT):
                    nr = min(RT, rows_x - rt)
                    n = nr * W
                    ps = pp.tile([128, n], F32, tag="cvps", bufs=2,
                                 name=f"ps{k}")
                    ti = 0
                    for ki in range(ksz):
                        for kj in range(ksz):
                            dy, dx = ki - pad, kj - pad
                            tap = TAP_OFF[k] + ki * ksz + kj
                            r0 = HALO - d + rt + dy
                            nc.tensor.matmul(
                                ps[:, :],
                                lhsT=wtap_s[:, tap * 128:tap * 128 + 128],
                                rhs=x0v[:, r0:r0 + nr, 3 + dx:3 + dx + W],
                                start=(ti == 0), stop=(ti == ksz * ksz - 1))
                            ti += 1
                    nc.scalar.activation(
                        xv[:, rt:rt + nr, 2 * d:2 * d + W],
                        ps[:, :].rearrange("p (r w) -> p r w", w=W),
                        ACTF.Identity, bias=cb_s[:, k:k + 1])
                # zero image-pad rows of A half: x rows 0..d-1
                nc.gpsimd.memset(xk[0:64, 0:d * Wx], 0.0)
                if dbg:
                    nc.sync.dma_start(out=dbg_x[k][:, :], in_=xk[:, :])

                # ---- post stage: blocks of RB owned rows
                WPK = W + 2 * d
                vs = [(-d, -d), (-d, 0), (-d, d), (0, -d)]
                for b in range(0, OWNH, RB):
                    ts = []
                    for j, (dy, dx) in enumerate(vs):
                        o1 = o1p.tile([128, (RB + d) * WPK], BF16, tag="o1",
                                      name=f"o1_{k}_{b}_{j}")
                        o1v = o1[:, :].rearrange("p (r w) -> p r w", w=WPK)
                        xr0 = d + b   # x row of o1 row 0
                        nc.vector.tensor_sub(
                            o1v[:, :, :],
                            xv[:, xr0:xr0 + RB + d, d:d + WPK],
                            xv[:, xr0 + dy:xr0 + dy + RB + d,
                               d + dx:d + dx + WPK])
                        tj = tp.tile([128, RB * W], BF16, tag=f"t{j}",
                                     name=f"t{k}_{b}_{j}")
                        nc.vector.tensor_mul(
                            tj[:, :].rearrange("p (r w) -> p r w", w=W),
                            o1v[:, 0:RB, d:d + W],
                            o1v[:, -dy:-dy + RB, d - dx:d - dx + W])
                        ts.append(tj)
                    t0_, t1_, t2_, t3_ = [t[:, :] for t in ts]
                    e1 = tp.tile([128, RB * W], BF16, tag="e1",
                                 name=f"e{k}_{b}")[:, :]
                    # 5-comparator sort network (ascending finals:
                    # t3_=o(1), t1_=o(2), t0_=o(3), e1=o(4))
                    nc.vector.tensor_tensor(e1, t0_, t1_, ALU.max)
                    nc.vector.tensor_tensor(t0_, t0_, t1_, ALU.min)
                    nc.vector.tensor_tensor(t1_, t2_, t3_, ALU.max)
                    nc.vector.tensor_tensor(t2_, t2_, t3_, ALU.min)
                    nc.vector.tensor_tensor(t3_, t0_, t2_, ALU.min)
                    nc.vector.tensor_tensor(t0_, t0_, t2_, ALU.max)
                    nc.vector.tensor_tensor(t2_, e1, t1_, ALU.min)
                    nc.vector.tensor_tensor(e1, e1, t1_, ALU.max)
                    nc.vector.tensor_tensor(t1_, t0_, t2_, ALU.min)
                    nc.vector.tensor_tensor(t0_, t0_, t2_, ALU.max)
                    # weighted sum with adjusted scales
                    y = yp.tile([128, RB * W], BF16, tag="y",
                                name=f"y{k}_{b}")[:, :]
                    nc.vector.tensor_scalar_mul(y, t3_, sadj_s[:, 4 * k:4 * k + 1])
                    nc.vector.scalar_tensor_tensor(
                        y, t1_, sadj_s[:, 4 * k + 1:4 * k + 2], y,
                        op0=ALU.mult, op1=ALU.add)
                    nc.vector.scalar_tensor_tensor(
                        y, t0_, sadj_s[:, 4 * k + 2:4 * k + 3], y,
                        op0=ALU.mult, op1=ALU.add)
                    nc.vector.scalar_tensor_tensor(
                        y, e1, sadj_s[:, 4 * k + 3:4 * k + 4], y,
                        op0=ALU.mult, op1=ALU.add)
                    vmx = vmax[:, b * W:(b + RB) * W]
                    vsm = vsum[:, b * W:(b + RB) * W]
                    if k == 0:
                        nc.vector.tensor_copy(vmx, y)
                        nc.vector.tensor_copy(vsm, y)
                    else:
                        nc.vector.tensor_tensor(vmx, vmx, y, ALU.max)
                        nc.vector.tensor_tensor(vsm, vsm, y, ALU.add)

            if dbg:
                nc.sync.dma_start(out=dbg_v[0][:, :], in_=vmax[:, :])
                nc.sync.dma_start(out=dbg_v[1][:, :], in_=vsum[:, :])

            # ---- final stage: blocks of RF rows per half
            for h in range(2):
                for f in range(0, OWNH, RF):
                    n = RF * W
                    sl = slice((h * 0 + f) * W, (f + RF) * W)
                    mt = fp.tile([128, n], BF16, tag="mt", name="mt")[:, :]
                    nc.vector.scalar_tensor_tensor(
                        mt, vsum[:, sl], 0.25, vmax[:, sl],
                        op0=ALU.mult, op1=ALU.add)
                    mr = fp.tile([128, n], BF16, tag="mr", name="mr")[:, :]
                    nc.scalar.activation(mr, mt, ACTF.Relu)
                    zps = pp.tile([128, n], F32, tag="zps", bufs=2,
                                  name="zps")
                    nc.tensor.matmul(zps[:, :], lhsT=wbc_s[:, :], rhs=mr,
                                     start=True, stop=True)
                    # BN + SiLU: silu(v) = v * sigmoid(v), v = scale*z + bias
                    zlin = fp.tile([128, n], BF16, tag="zlin", name="zlin")[:, :]
                    nc.scalar.activation(zlin, zps[:, :], ACTF.Identity,
                                         bias=bnbi_s[:, 0:1],
                                         scale=bnsc_s[:, 0:1])
                    zsig = fp.tile([128, n], BF16, tag="zsig", name="zsig")[:, :]
                    nc.scalar.activation(zsig, zps[:, :], ACTF.Sigmoid,
                                         bias=bnbi_s[:, 0:1],
                                         scale=bnsc_s[:, 0:1])
                    zt = fp.tile([128, n], BF16, tag="zt", name="zt")[:, :]
                    nc.vector.tensor_mul(zt, zlin, zsig)
                    lps = pp.tile([1, n], F32, tag=f"lps{h}", bufs=1,
                                  name="lps")
                    nc.tensor.matmul(lps[:, :], lhsT=wfc_s[:, h:h + 1],
                                     rhs=zt, start=True, stop=True)
                    msk = fp.tile([1, n], BF16, tag="msk", name="msk")
                    nc.scalar.activation(msk[:, :], lps[:, :], ACTF.Sigmoid,
                                         bias=fcb_s[0:1, 0:1])
                    mb = pp.tile([128, n], F32, tag="mb", bufs=2, name="mb")
                    nc.tensor.matmul(mb[:, :], lhsT=ones_s[:, :],
                                     rhs=msk[:, :], start=True, stop=True)
                    for c in range(2):
                        cent = fp.tile([128, n], F32, tag=f"cf{c}",
                                       name="cent")
                        src = (HALO + h * OWNH + f) * W
                        nc.gpsimd.dma_start(out=cent[:, :],
                                          in_=cen[c][:, src:src + n])
                        ot = fp.tile([128, n], F32, tag=f"ot{c}", name="ot")
                        nc.vector.scalar_tensor_tensor(
                            ot[:, :], mb[:, :], 1.0, cent[:, :],
                            op0=ALU.add, op1=ALU.mult)
                        dst = (h * OWNH + f) * W
                        nc.gpsimd.dma_start(out=outs[c][:, dst:dst + n],
                                          in_=ot[:, :])
    nc.compile()
    nc.finalize()
    return nc


_NC_CACHE = None


def _get_nc():
    global _NC_CACHE
    if _NC_CACHE is None:
        _NC_CACHE = build_nc()
    return _NC_CACHE


def _prep_core_inputs(cen_b, flip, wts):
    """cen_b: (256, 160, 160) fp32 for this batch; flip: bottom half?"""
    (w_in, b_in, convs, scales_adj, bc_w, bn_scale, bn_bias,
     fc_w, fc_b) = wts
    lo = (1 if flip else 0) * 80 - HALO
    sh = np.zeros((C, SH, W), np.float32)
    r0, r1 = max(0, lo), min(160, lo + SH)
    sh[:, r0 - lo:r1 - lo] = cen_b[:, r0:r1]
    if flip:
        sh = sh[:, ::-1]
    sh = np.ascontiguousarray(sh)

    bf = ml_dtypes.bfloat16
    wtap = np.zeros((128, NTAP * 128), bf)
    for k in range(4):
        ksz = KS[k]
        cw = convs[k][0]
        if flip:
            cw = cw[:, :, ::-1, :]
        for ki in range(ksz):
            for kj in range(ksz):
                t = TAP_OFF[k] + ki * ksz + kj
                blk = cw[:, :, ki, kj].T.astype(bf)  # [ci, co]
                wtap[0:64, t * 128:t * 128 + 64] = blk
                wtap[64:128, t * 128 + 64:t * 128 + 128] = blk

    win = np.zeros((128, 128), bf)
    win[:, 0:64] = w_in[:, 0:128].T.astype(bf)
    win[:, 64:128] = w_in[:, 128:256].T.astype(bf)

    wbc = np.zeros((128, 128), bf)
    wbc[0:64, 0:64] = bc_w.T.astype(bf)
    wbc[64:128, 64:128] = bc_w.T.astype(bf)

    wfc = np.zeros((128, 2), bf)
    wfc[0:64, 0] = fc_w.astype(bf)
    wfc[64:128, 1] = fc_w.astype(bf)

    dup = lambda v: np.concatenate([v, v]).astype(np.float32).reshape(128, -1)
    m = {
        "cen0": sh[0:128].reshape(128, SH * W),
        "cen1": sh[128:256].reshape(128, SH * W),
        "cenb0": sh[0:128].reshape(128, SH * W).astype(bf),
        "cenb1": sh[128:256].reshape(128, SH * W).astype(bf),
        "win": win,
        "wtap": wtap,
        "wbc": wbc,
        "wfc": wfc,
        "ones1": np.ones((1, 128), bf),
    "bin": dup(b_in),
        "cb": np.concatenate([np.stack([cb for _, cb in convs], 1)] * 2, 0)
              .astype(np.float32),
        "sadj": np.concatenate([scales_adj.reshape(64, 16)] * 2, 0)
                .astype(np.float32),
        "bnsc": dup(bn_scale),
        "bnbi": dup(bn_bias),
        "fcb": np.full((1, 1), fc_b, np.float32),
    }
    return m


def make_in_maps(inputs):
    cen = np.asarray(inputs["cen"], np.float32)
    w_in = np.asarray(inputs["in_conv_w"], np.float32).reshape(CH, C)
    convs = [(np.asarray(inputs[f"conv{k}_w"], np.float32),
              np.asarray(inputs[f"conv{k}_b"], np.float32))
             for k in (1, 3, 5, 7)]
    # s'[g, j] = -scales3[g, 3-j]
    sadj = -np.asarray(inputs["scales3"], np.float32)[:, ::-1]    # (64, 4)
    sadj4 = np.repeat(sadj[:, None, :], 4, axis=1)                # (64, 4, 4)
    bn_scale = (np.asarray(inputs["bn_gamma"]) /
                np.sqrt(np.asarray(inputs["bn_var"]) + 1e-5)).astype(np.float32)
    bn_bias = (np.asarray(inputs["bn_beta"]) -
               np.asarray(inputs["bn_mean"]) * bn_scale).astype(np.float32)
    wts = (w_in, np.asarray(inputs["in_conv_b"], np.float32), convs, sadj4,
           np.asarray(inputs["bc_w"], np.float32).reshape(CH, CH),
           bn_scale, bn_bias,
           np.asarray(inputs["fc_w"], np.float32).reshape(CH),
           float(np.asarray(inputs["fc_b"])[0]))
    in_maps = []
    for core in range(8):
        b, half = core // 2, core % 2
        in_maps.append(_prep_core_inputs(cen[b], half == 1, wts))
    return in_maps


def kernel(**inputs):
    in_maps = make_in_maps(inputs)
    nc = _get_nc()
    res = bass_utils.run_bass_kernel_spmd(nc, in_maps,
                                          core_ids=list(range(8)))
    out = np.empty((4, C, 160, W), np.float32)
    for core in range(8):
        b, half = core // 2, core % 2
        o = np.concatenate(
            [res.results[core]["out0"].reshape(128, 2 * OWNH, W),
             res.results[core]["out1"].reshape(128, 2 * OWNH, W)], 0)
        # rows: [A(40) | B(40)] in flipped-shard coords
        if half == 1:
            o = o[:, ::-1]
        out[b, :, half * 80:(half + 1) * 80] = o
    return out

